# revision 61
# baseline (speedup 1.0000x reference)
"""Trainium2 Bass kernel for GQA causal attention (B=2, S=2048, D=2048,
16 q-heads / 4 kv-heads, head_dim=128, interleaved RoPE).

Sharding: DP=2 over batch x TP=4 over head groups (8 cores).
Core c: batch b=c//4, rank r=c%4 -> q-heads [4r,4r+4), kv-head r.

v2: chunk-major software pipeline. Projections (K,V,Q) for chunk j+1 and
the strip-0 output projection are emitted as PE filler inside attention
chunk j, so the PE never waits on softmax (Act/DVE) work. DVE load cut
~3x: rope via stream_shuffle (even/odd packed at 16-row granularity in
each 32-partition quadrant) + 3 bf16 ops; softmax denominator
accumulated in bf16 split across DVE (masked tiles) and Pool (unmasked);
reciprocal via reciprocal_approx_fast. Host-side work is layout only:
slicing, transposing, bf16 casting.
"""

import math
import sys

sys.path.insert(0, "/opt/trn_rl_repo")

from contextlib import ExitStack

import ml_dtypes
import numpy as np

import concourse.bass as bass
import concourse.mybir as mybir
import concourse.tile as tile
from concourse import bacc
from concourse.bass_utils import run_bass_kernel_spmd
from concourse.masks import make_identity

BF16 = mybir.dt.bfloat16
F32 = mybir.dt.float32

N_HEADS = 16
N_KV_HEADS = 4
HD = 128
ROPE_THETA = 10000.0
TP = 4
N_CORES = 8

SWAP16 = [(i + 16) % 32 for i in range(32)]  # quadrant half-swap


def build_graph(S=2048, D=2048, HQL=4, NS=512):
    """Per-core SPMD graph. HQL = local q heads; local kv heads = 1."""
    hd = HD
    ND = D // 128          # d-tiles (projection contraction tiles)
    NC = S // NS           # s-chunks
    NK = S // 128          # sk-tiles
    MQ = HQL * hd          # local q width
    DIAG = NS // 128       # sk-tiles per chunk needing a causal mask
    NB = N_CORES // TP     # batches
    OW = S // N_CORES      # out cols per core per batch
    NM = max(1, S // (N_CORES * 128))   # strips (AllToAll count)
    SW = OW // NM          # strip width (=128 at full size)
    per = NC // NM         # chunks per strip
    scale = 1.0 / math.sqrt(hd)
    NH = TP * HQL          # global head count
    NO = D // NS           # out-proj n tiles

    nc = bacc.Bacc("TRN2", target_bir_lowering=False, debug=False,
                   num_devices=N_CORES)

    # All weight/activation inputs are host-packed partition-major
    # ([128, tiles*width]) so each loads with one large-row DMA.
    xT_e = nc.dram_tensor("xT", [128, ND, S], BF16,
                          kind="ExternalInput").ap()
    wqT_e = nc.dram_tensor("wqT", [128, ND * MQ], BF16,
                           kind="ExternalInput").ap()
    wkT_e = nc.dram_tensor("wkT", [128, ND * hd], BF16,
                           kind="ExternalInput").ap()
    wvT_e = nc.dram_tensor("wvT", [128, ND * hd], BF16,
                           kind="ExternalInput").ap()
    woT_e = nc.dram_tensor("woT", [128, NH * D], BF16,
                           kind="ExternalInput").ap()
    cc_e = nc.dram_tensor("cc", [128, S], BF16, kind="ExternalInput").ap()
    ss_e = nc.dram_tensor("ss", [128, S], BF16, kind="ExternalInput").ap()
    mask_e = nc.dram_tensor("mask", [128, NS + (DIAG - 1) * 128], BF16,
                            kind="ExternalInput").ap()
    # bf16 staging for the output (host casts back to f32): halves the
    # tail out-DMA bytes; adds ~0.4% element-wise rounding, well inside
    # the error budget
    out_e = nc.dram_tensor("out", [NB * OW, D], BF16,
                           kind="ExternalOutput").ap()

    # a2a payload: per-dest block = [128 e-rows, HG*SW] from each source
    # core (source (b, r) holds heads r*HQL..r*HQL+HQL-1 of batch b).
    # Each strip's exchange is split into two head-half collectives so
    # the first can rendezvous + transfer while the second half's
    # attention passes still run.
    HG = 1                  # heads per collective
    NG = HQL // HG          # collectives per strip
    a2a_in = [[nc.dram_tensor(f"a2a_in{m}_{g}",
                              [N_CORES * 128, HG * SW], BF16)
               for g in range(NG)] for m in range(NM)]
    a2a_out = [[nc.dram_tensor(f"a2a_out{m}_{g}",
                               [N_CORES * 128, HG * SW], BF16)
                for g in range(NG)] for m in range(NM)]
    groups = [list(range(N_CORES))]

    with tile.TileContext(nc) as tc, ExitStack() as ctx:
        ep = ctx.enter_context
        const_pool = ep(tc.tile_pool(name="const", bufs=1))
        rt_pool = ep(tc.tile_pool(name="rt", bufs=HQL + 1))
        vnat_pool = ep(tc.tile_pool(name="vnat", bufs=1))
        vst_pool = ep(tc.tile_pool(name="vst", bufs=2))
        xt_pool = ep(tc.tile_pool(name="xt", bufs=2))
        wq_pool = ep(tc.tile_pool(name="wq", bufs=1))
        wkv_pool = ep(tc.tile_pool(name="wkv", bufs=1))
        wo_pool = ep(tc.tile_pool(name="wo", bufs=1))
        stg_pool = ep(tc.tile_pool(name="stg", bufs=3))
        sw_pool = ep(tc.tile_pool(name="sw", bufs=3))
        rtmp_pool = ep(tc.tile_pool(name="rtmp", bufs=4))
        pt_pool = ep(tc.tile_pool(name="pt", bufs=5))
        den_pool = ep(tc.tile_pool(name="den", bufs=2))
        rc_pool = ep(tc.tile_pool(name="rc", bufs=2))
        rbc_pool = ep(tc.tile_pool(name="rbc", bufs=2))
        attn_pool = ep(tc.tile_pool(name="attn", bufs=3))
        ao_pool = ep(tc.tile_pool(name="ao", bufs=2 * TP * 2))
        osb_pool = ep(tc.tile_pool(name="osb", bufs=3))
        # PSUM: 8 banks of [128, 512]f32. proj/outproj 2 (disjoint in
        # time) + score/transpose 2 + attention-out 2 + den-sum 2 =
        # 16KB/partition exactly.
        proj_ps = ep(tc.tile_pool(name="pops", bufs=2, space="PSUM"))
        op_ps = proj_ps
        sc_ps = ep(tc.tile_pool(name="scps", bufs=3, space="PSUM"))
        at_ps_pool = ep(tc.tile_pool(name="atps", bufs=2, space="PSUM"))
        dps_pool = ep(tc.tile_pool(name="dps", bufs=1, space="PSUM"))

        # ---- constants ----
        ident = const_pool.tile([128, 128], BF16, tag="ident")
        make_identity(nc, ident[:])
        ones = const_pool.tile([128, 2], BF16, tag="ones")
        nc.gpsimd.memset(ones[:], 1.0)
        cc = const_pool.tile([128, S], BF16, tag="cc")
        ss = const_pool.tile([128, S], BF16, tag="ss")
        msk = const_pool.tile([128, NS + (DIAG - 1) * 128], BF16, tag="msk")

        def dma_rope_consts():
            for q in range(4):
                w = S // 4
                nc.sync.dma_start(cc[:, q * w:(q + 1) * w],
                                  cc_e[:, q * w:(q + 1) * w])
                nc.sync.dma_start(ss[:, q * w:(q + 1) * w],
                                  ss_e[:, q * w:(q + 1) * w])
            nc.sync.dma_start(msk[:], mask_e[:])

        # ---- weight prefetch (sync queue, one big DMA per tensor) ----
        xts = {}  # (d, j) -> AP view

        NSPLIT = max(1, ND // 8)  # d-tiles per DMA: parallel DMA engines

        def dma_x_chunk(j):
            t = xt_pool.tile([128, ND, NS], BF16, tag="xt",
                             name=f"xc{j}")
            for d0 in range(0, ND, NSPLIT):
                d1 = d0 + NSPLIT
                nc.sync.dma_start(t[:, d0:d1, :],
                                  xT_e[:, d0:d1, j * NS:(j + 1) * NS])
            for d in range(ND):
                xts[(d, j)] = t[:, d, :]

        def dma_w(dst, src, width, nsp=4):
            step = (width + nsp - 1) // nsp
            for c0 in range(0, width, step):
                c1 = min(width, c0 + step)
                nc.sync.dma_start(dst[:, c0:c1], src[:, c0:c1])

        wk_all = wkv_pool.tile([128, ND * hd], BF16, tag="wk")
        dma_w(wk_all, wkT_e, ND * hd)
        wks = [wk_all[:, d * hd:(d + 1) * hd] for d in range(ND)]
        dma_x_chunk(0)
        dma_rope_consts()
        wv_all = wkv_pool.tile([128, ND * hd], BF16, tag="wv")
        dma_w(wv_all, wvT_e, ND * hd)
        wvs = [wv_all[:, d * hd:(d + 1) * hd] for d in range(ND)]
        wq_all = wq_pool.tile([128, ND * MQ], BF16, tag="wq")
        dma_w(wq_all, wqT_e, ND * MQ, nsp=8)
        wqs = [wq_all[:, d * MQ:(d + 1) * MQ] for d in range(ND)]
        dma_x_chunk(1)

        # persistent attention operand tiles
        rts = [rt_pool.tile([128, S], BF16, tag="rt", name=f"rtq{h}")
               for h in range(HQL)]
        krt = rt_pool.tile([128, S], BF16, tag="rt", name="rtk")
        vnat = vnat_pool.tile([128, S], BF16, tag="vnat")
        wo_all = wo_pool.tile([128, NH * D], BF16, tag="wo")
        wo_tiles = [wo_all[:, ht * D:(ht + 1) * D] for ht in range(NH)]

        # ---- emitters ----
        def rope_from_ps(ps, dst, ssl):
            """RoPE a projected [128, NS] psum tile into dst[:, ssl] (bf16).

            Rows packed per 32-quadrant: [even pairs 16b..16b+15,
            odd pairs 16b..16b+15]; swap = quadrant half-rotation.
            """
            stg = stg_pool.tile([128, NS], BF16, tag="stg")
            nc.scalar.copy(stg[:], ps[:])
            sw = sw_pool.tile([128, NS], BF16, tag="sw")
            nc.vector.stream_shuffle(sw[:], stg[:], SWAP16)
            t1 = rtmp_pool.tile([128, NS], BF16, tag="rtmp")
            nc.vector.tensor_mul(t1[:], stg[:], cc[:, ssl])
            t2 = rtmp_pool.tile([128, NS], BF16, tag="rtmp")
            nc.vector.tensor_mul(t2[:], sw[:], ss[:, ssl])
            nc.vector.tensor_add(dst[:, ssl], t1[:], t2[:])

        def proj_tile_emitters(lhs_tiles, mslice, j, kind, dst):
            """One [128, NS] projection tile split into 4-matmul quarter
            emitters + a tail (rope/copy), so filler interleaves finely
            and the Act exp stream never starves behind a PE block."""
            ssl = slice(j * NS, (j + 1) * NS)
            state = {}

            def quarter(d0):
                if 'ps' not in state:
                    state['ps'] = proj_ps.tile([128, NS], F32,
                                               tag="pops", name="psp")
                ps = state['ps']
                for d in range(d0, min(d0 + 4, ND)):
                    nc.tensor.matmul(ps[:], lhs_tiles[d][:, mslice],
                                     xts[(d, j)][:, :],
                                     start=(d == 0), stop=(d == ND - 1))

            def tail():
                ps = state['ps']
                if kind == "v":
                    vst = vst_pool.tile([128, NS], BF16, tag="vst")
                    nc.scalar.copy(vst[:], ps[:])
                    for t in range(DIAG):
                        tpp = sc_ps.tile([128, 128], BF16, tag="scps",
                                         name="pst")
                        nc.tensor.transpose(
                            tpp[:], vst[:, t * 128:(t + 1) * 128],
                            ident[:])
                        nc.scalar.copy(
                            vnat[:, (j * DIAG + t) * 128:
                                 (j * DIAG + t + 1) * 128], tpp[:])
                else:
                    rope_from_ps(ps, dst, ssl)

            ems = [lambda d0=d0: quarter(d0) for d0 in range(0, ND, 4)]
            ems.append(tail)
            return ems

        def proj_chunk_emitters(j):
            ems = []
            if j >= 1 and j + 1 < NC:
                ems.append(lambda j=j: dma_x_chunk(j + 1))
            ems += proj_tile_emitters(wks, slice(0, hd), j, "k", krt)
            ems += proj_tile_emitters(wvs, slice(0, hd), j, "v", None)
            for h in range(HQL):
                ems += proj_tile_emitters(
                    wqs, slice(h * hd, (h + 1) * hd), j, "q", rts[h])
            return ems

        def wo_dma_emitters():
            def one(q):  # eighth-sized DMAs: parallel engines + pipelining
                w = NH * D // 8
                nc.sync.dma_start(wo_all[:, q * w:(q + 1) * w],
                                  woT_e[:, q * w:(q + 1) * w])
            return [lambda q=q: one(q) for q in range(8)]

        # ---- attention ----
        class Pass:
            def __init__(self, j, h):
                self.j = j
                self.h = h
                self.nsk = (j + 1) * DIAG
                self.at_ps = None
                self.den = None

        def score_stage(p, si):
            # Diagonal tiles only have valid data for q >= o; narrow all
            # work (score/exp/mask/den/AV) to the live column range.
            o = max(0, si * 128 - p.j * NS)
            w = NS - o
            sc = sc_ps.tile([128, NS], F32, tag="scps", name="psc")
            nc.tensor.matmul(
                sc[:, 0:w], krt[:, si * 128:(si + 1) * 128],
                rts[p.h][:, p.j * NS + o:(p.j + 1) * NS],
                start=True, stop=True)
            pt = pt_pool.tile([128, NS], BF16, tag="pt")
            nc.scalar.activation(
                pt[:, 0:w], sc[:, 0:w],
                mybir.ActivationFunctionType.Exp, scale=scale)
            if si * 128 - p.j * NS >= 0:  # diagonal block: causal mask
                nc.vector.tensor_mul(
                    pt[:, 0:w], pt[:, 0:w], msk[:, NS - 128:2 * NS - 128 - o])
            return pt

        def denav_stage(p, si, pt):
            o = max(0, si * 128 - p.j * NS)
            w = NS - o
            if p.at_ps is None:
                p.at_ps = at_ps_pool.tile(
                    [128, NS], F32, tag="atps",
                    name=f"atps_j{p.j}_h{p.h}")
            # denominator accumulates on the DVE in bf16 (PE stays free);
            # one ones-matmul per pass does the partition sum
            if p.den is None:
                p.den = den_pool.tile([128, NS], BF16, tag="den",
                                      name=f"den_j{p.j}_h{p.h}")
                nc.vector.tensor_copy(p.den[:], pt[:])
            else:
                nc.vector.tensor_add(p.den[:, o:NS], p.den[:, o:NS],
                                     pt[:, 0:w])
            last = si == p.nsk - 1
            nc.tensor.matmul(
                p.at_ps[:, o:NS], vnat[:, si * 128:(si + 1) * 128],
                pt[:, 0:w], start=(si == 0), stop=last)

        def finalize_pass(p):
            # runs 2 pipeline steps after the pass's last denav so the
            # PE-queue den matmul never waits on the DVE's den backlog
            dps = dps_pool.tile([1, NS], F32, tag="dps",
                                name=f"dps_j{p.j}_h{p.h}")
            nc.tensor.matmul(dps[:], ones[:, 0:1], p.den[:],
                             start=True, stop=True)
            rc = rc_pool.tile([1, NS], F32, tag="rc")
            nc.vector.reciprocal_approx_fast(out=rc[:], in_=dps[:])
            rbc = rbc_pool.tile([128, NS], F32, tag="rbc")
            nc.gpsimd.partition_broadcast(rbc[:], rc[:])
            p.rbc = rbc

        def normalize_export(p):
            asb = attn_pool.tile([128, NS], BF16, tag="attn")
            nc.vector.tensor_mul(asb[:], p.at_ps[:], p.rbc[:])
            g, hh = p.h // HG, p.h % HG
            for i in range(NS // SW):
                c = p.j * NS + i * SW
                dd = (c // SW) % N_CORES
                m = c // (N_CORES * SW)
                nc.gpsimd.dma_start(
                    a2a_in[m][g].ap()[dd * 128:(dd + 1) * 128,
                                      hh * SW:(hh + 1) * SW],
                    asb[:, i * SW:(i + 1) * SW])

        def do_a2a(m, g):
            # the CC transfer runs async on the CC cores; the Pool queue
            # keeps flowing past this instruction
            nc.gpsimd.collective_compute(
                "AllToAll", mybir.AluOpType.bypass,
                ins=[a2a_in[m][g].ap().opt()],
                outs=[a2a_out[m][g].ap().opt()],
                replica_groups=groups)

        # ---- output projection ----
        def outproj_emitters(m):
            ems = []

            def mk(m, beta):
                state = {'pso': {}}

                def do_imports():
                    # aos[(h, r)]: head-dim block of global head r*HQL+h
                    aos = {}
                    for g in range(NG):
                        for r in range(TP):
                            src = beta * TP + r
                            t = ao_pool.tile(
                                [128, HG * SW], BF16, tag="ao",
                                name=f"ao_{m}_{beta}_{r}_{g}")
                            nc.sync.dma_start(
                                t[:], a2a_out[m][g].ap()
                                [src * 128:(src + 1) * 128, :])
                            for hh in range(HG):
                                aos[(g * HG + hh, r)] = \
                                    t[:, hh * SW:(hh + 1) * SW]
                    state['aos'] = aos

                def part(n, hs):
                    # matmuls for heads hs of output group n (h-major so
                    # early heads' work overlaps later heads' collectives)
                    if n not in state['pso']:
                        state['pso'][n] = op_ps.tile(
                            [SW, NS], F32, tag="pops",
                            name=f"pso_{m}_{beta}_{n}")
                    pso = state['pso'][n]
                    for h in hs:
                        for r in range(TP):
                            ht = r * HQL + h
                            nc.tensor.matmul(
                                pso[:], state['aos'][(h, r)][:],
                                wo_tiles[ht][:, n * NS:(n + 1) * NS],
                                start=(h == 0 and r == 0),
                                stop=(h == HQL - 1 and r == TP - 1))

                def fin(n):
                    part(n, [HQL - 1])
                    pso = state['pso'].pop(n)
                    ob = osb_pool.tile([SW, NS], BF16, tag="osb")
                    # split the staging copy across Act and DVE and the
                    # write across two DMAs: halves the exposed drain of
                    # the final group
                    hw_ = NS // 2
                    nc.scalar.copy(ob[:, 0:hw_], pso[:, 0:hw_])
                    nc.vector.tensor_copy(ob[:, hw_:NS], pso[:, hw_:NS])
                    r0 = beta * OW + m * SW
                    c0 = n * NS
                    nc.sync.dma_start(
                        out_e[r0:r0 + SW, c0:c0 + hw_], ob[:, 0:hw_])
                    nc.sync.dma_start(
                        out_e[r0:r0 + SW, c0 + hw_:c0 + NS], ob[:, hw_:NS])

                head = list(range(HQL - 1))
                ems.append(do_imports)
                # rolling schedule, <=2 psum groups alive: part(0) part(1)
                # fin(0) part(2) fin(1) part(3) fin(2) fin(3)
                for n in range(min(2, NO)):
                    ems.append(lambda n=n: part(n, head))
                for n in range(2, NO):
                    ems.append(lambda n=n: fin(n - 2))
                    ems.append(lambda n=n: part(n, head))
                for n in range(max(0, NO - 2), NO):
                    ems.append(lambda n=n: fin(n))

            for beta in range(NB):
                mk(m, beta)
            return ems

        # ---- top-level emission schedule ----
        # preamble: only K, V, Q-h0 of chunk 0; Q-h1..h3 become chunk-0
        # filler so softmax engines start ~3 proj tiles earlier
        for em in (proj_tile_emitters(wks, slice(0, hd), 0, "k", krt)
                   + proj_tile_emitters(wvs, slice(0, hd), 0, "v", None)
                   + proj_tile_emitters(wqs, slice(0, hd), 0, "q",
                                        rts[0])):
            em()
        q0_filler = []     # (emitter, needed_before_pass_h)
        for h in range(1, HQL):
            for em in proj_tile_emitters(
                    wqs, slice(h * hd, (h + 1) * hd), 0, "q", rts[h]):
                q0_filler.append((em, h))

        pending = []

        def flush_one():
            item = pending.pop(0)
            if item[0] == 'fin':
                pp = item[1]
                finalize_pass(pp)
                normalize_export(pp)
                if pp.j % per == per - 1 and (pp.h + 1) % HG == 0:
                    do_a2a(pp.j // per, (pp.h + 1) // HG - 1)
                return
            _, pp, psi, ppt = item
            denav_stage(pp, psi, ppt)
            if psi == pp.nsk - 1:
                pending.append(('fin', pp))

        def flush_pending():
            while pending:
                flush_one()

        for j in range(NC):
            # filler work to interleave into this chunk's attention.
            # late_filler depends on a collective result: emit it only in
            # the second half of the chunk so the in-order PE queue never
            # reaches it before the AllToAll lands.
            filler = []
            late_filler = []
            if j == 0:
                filler += [em for em, _ in q0_filler]
            if j + 1 < NC:
                filler += proj_chunk_emitters(j + 1)
            if j == 0:
                filler += wo_dma_emitters()
            for m in range(NM):
                if j >= (m + 1) * per + 1:
                    late_filler += outproj_emitters(m)
            steps = HQL * (j + 1) * DIAG
            half = steps // 2
            fper = len(filler) / max(1, half)
            lper = len(late_filler) / max(1, steps - half)
            facc = 0.0
            fidx = 0
            lacc = 0.0
            lidx = 0
            step = 0
            per_h = (len(q0_filler) // max(1, HQL - 1)) if j == 0 else 0
            for h in range(HQL):
                p = Pass(j, h)
                if j == 0 and h > 0:
                    # pass h's scores read rts[h]: its projection filler
                    # must be emitted first (trace-time dep ordering)
                    while fidx < h * per_h:
                        filler[fidx]()
                        fidx += 1
                    facc = max(facc, float(fidx))
                for si in range(p.nsk):
                    pt = score_stage(p, si)
                    while len(pending) >= 2:
                        flush_one()
                    pending.append(('step', p, si, pt))
                    step += 1
                    if step <= half:
                        facc += fper
                        while fidx < facc and fidx < len(filler):
                            filler[fidx]()
                            fidx += 1
                    else:
                        lacc += lper
                        while lidx < lacc and lidx < len(late_filler):
                            late_filler[lidx]()
                            lidx += 1
            while fidx < len(filler):
                filler[fidx]()
                fidx += 1
            while lidx < len(late_filler):
                late_filler[lidx]()
                lidx += 1
        flush_pending()

        # tail: strips whose out-projection wasn't emitted as filler
        for m in range(NM):
            if not (NC - 1 >= (m + 1) * per + 1):
                for em in outproj_emitters(m):
                    em()

    nc.compile()
    return nc


def host_prepare(x, wq, wk, wv, wo, S, D, HQL, NS):
    """Layout-only host prep: slice/transpose/cast + rope tables + mask."""
    hd = HD
    MQ = HQL * hd
    bf = ml_dtypes.bfloat16
    DIAG = NS // 128

    # head-dim permutation: per 32-row quadrant b, rows [16 even pairs,
    # 16 odd pairs] so stream_shuffle's quadrant half-swap exchanges them
    perm = np.empty(hd, dtype=np.int64)
    for r in range(hd):
        b, w = r // 32, r % 32
        perm[r] = 2 * (16 * b + w) if w < 16 else 2 * (16 * b + (w - 16)) + 1

    def permute_heads(w):
        nh = w.shape[0] // hd
        w = w.reshape(nh, hd, -1)[:, perm, :]
        return w.reshape(nh * hd, -1)

    wq_p = permute_heads(wq)
    wk_p = permute_heads(wk)

    inv_freq = 1.0 / (ROPE_THETA ** (np.arange(0, hd, 2, dtype=np.float64)
                                     / hd))
    ang = np.arange(S, dtype=np.float64)[None, :] * inv_freq[:, None]
    cosf = np.cos(ang)  # [hd/2, S], row = pair index
    sinf = np.sin(ang)
    cci = np.empty((128, S), dtype=np.float64)
    ssi = np.empty((128, S), dtype=np.float64)
    for r in range(128):
        b, w = r // 32, r % 32
        pair = 16 * b + (w % 16)
        cci[r] = cosf[pair]
        ssi[r] = -sinf[pair] if w < 16 else sinf[pair]
    cci = cci.astype(bf)
    ssi = ssi.astype(bf)

    p = np.arange(128)[:, None]
    c = np.arange(NS + (DIAG - 1) * 128)[None, :]
    mski = (p <= c - (NS - 128)).astype(bf)

    def pack(wT):
        """[D, W] -> [128, (D//128)*W] partition-major (one-DMA layout)."""
        Dd, W = wT.shape
        return np.ascontiguousarray(
            wT.reshape(Dd // 128, 128, W).transpose(1, 0, 2)
            .reshape(128, -1)).astype(bf)

    woT = pack(wo.T)

    in_maps = []
    for core in range(N_CORES):
        b = core // TP
        r = core % TP
        qsl = slice(r * MQ, (r + 1) * MQ)
        ksl = slice(r * hd, (r + 1) * hd)
        in_maps.append({
            "xT": pack(x[b].T).reshape(128, D // 128, S),
            "wqT": pack(wq_p[qsl].T),
            "wkT": pack(wk_p[ksl].T),
            "wvT": pack(wv[ksl].T),
            "woT": woT,
            "cc": cci, "ss": ssi, "mask": mski,
        })
    return in_maps


_NC_CACHE = {}


def get_graph(S=2048, D=2048, HQL=4, NS=512):
    key = (S, D, HQL, NS)
    if key not in _NC_CACHE:
        _NC_CACHE[key] = build_graph(S, D, HQL, NS)
    return _NC_CACHE[key]


def unshard_out(results, B, S, D):
    """results[core]["out"] is [NB*OW, D] with rows (beta, strip m, SW)."""
    out = np.empty((B, S, D), dtype=np.float32)
    OW = S // N_CORES
    NM = max(1, S // (N_CORES * 128))
    SW = OW // NM
    for core in range(N_CORES):
        r = results[core]["out"].astype(np.float32)
        for beta in range(B):
            for m in range(NM):
                c0 = core * SW + m * N_CORES * SW
                out[beta, c0:c0 + SW, :] = \
                    r[beta * OW + m * SW:beta * OW + (m + 1) * SW, :]
    return out


def kernel(x, wq, wk, wv, wo, trace=False):
    B, S, D = x.shape
    HQL = (wq.shape[0] // HD) // TP
    NS = 512
    nc = get_graph(S, D, HQL, NS)
    in_maps = host_prepare(x, wq, wk, wv, wo, S, D, HQL, NS)
    res = run_bass_kernel_spmd(nc, in_maps, core_ids=list(range(N_CORES)),
                               trace=trace)
    out = unshard_out(res.results, B, S, D)
    if trace:
        kernel.last_exec_time_ns = res.exec_time_ns
        kernel.last_results = res
    return out


# revision 65
# speedup vs baseline: 1.0097x; 1.0097x over previous
"""Trainium2 Bass kernel for GQA causal attention (B=2, S=2048, D=2048,
16 q-heads / 4 kv-heads, head_dim=128, interleaved RoPE).

Sharding: DP=2 over batch x TP=4 over head groups (8 cores).
Core c: batch b=c//4, rank r=c%4 -> q-heads [4r,4r+4), kv-head r.

v2: chunk-major software pipeline. Projections (K,V,Q) for chunk j+1 and
the strip-0 output projection are emitted as PE filler inside attention
chunk j, so the PE never waits on softmax (Act/DVE) work. DVE load cut
~3x: rope via stream_shuffle (even/odd packed at 16-row granularity in
each 32-partition quadrant) + 3 bf16 ops; softmax denominator
accumulated in bf16 split across DVE (masked tiles) and Pool (unmasked);
reciprocal via reciprocal_approx_fast. Host-side work is layout only:
slicing, transposing, bf16 casting.
"""

import math
import sys

sys.path.insert(0, "/opt/trn_rl_repo")

from contextlib import ExitStack

import ml_dtypes
import numpy as np

import concourse.bass as bass
import concourse.mybir as mybir
import concourse.tile as tile
from concourse import bacc
from concourse.bass_utils import run_bass_kernel_spmd
from concourse.masks import make_identity

BF16 = mybir.dt.bfloat16
F32 = mybir.dt.float32

N_HEADS = 16
N_KV_HEADS = 4
HD = 128
ROPE_THETA = 10000.0
TP = 4
N_CORES = 8

SWAP16 = [(i + 16) % 32 for i in range(32)]  # quadrant half-swap


def build_graph(S=2048, D=2048, HQL=4, NS=512):
    """Per-core SPMD graph. HQL = local q heads; local kv heads = 1."""
    hd = HD
    ND = D // 128          # d-tiles (projection contraction tiles)
    NC = S // NS           # s-chunks
    NK = S // 128          # sk-tiles
    MQ = HQL * hd          # local q width
    DIAG = NS // 128       # sk-tiles per chunk needing a causal mask
    NB = N_CORES // TP     # batches
    OW = S // N_CORES      # out cols per core per batch
    NM = max(1, S // (N_CORES * 128))   # strips (AllToAll count)
    SW = OW // NM          # strip width (=128 at full size)
    per = NC // NM         # chunks per strip
    scale = 1.0 / math.sqrt(hd)
    NH = TP * HQL          # global head count
    NO = D // NS           # out-proj n tiles

    nc = bacc.Bacc("TRN2", target_bir_lowering=False, debug=False,
                   num_devices=N_CORES)

    # All weight/activation inputs are host-packed partition-major
    # ([128, tiles*width]) so each loads with one large-row DMA.
    xT_e = nc.dram_tensor("xT", [128, ND, S], BF16,
                          kind="ExternalInput").ap()
    wqT_e = nc.dram_tensor("wqT", [128, ND * MQ], BF16,
                           kind="ExternalInput").ap()
    wkT_e = nc.dram_tensor("wkT", [128, ND * hd], BF16,
                           kind="ExternalInput").ap()
    wvT_e = nc.dram_tensor("wvT", [128, ND * hd], BF16,
                           kind="ExternalInput").ap()
    woT_e = nc.dram_tensor("woT", [128, NH * D], BF16,
                           kind="ExternalInput").ap()
    cc_e = nc.dram_tensor("cc", [128, S], BF16, kind="ExternalInput").ap()
    ss_e = nc.dram_tensor("ss", [128, S], BF16, kind="ExternalInput").ap()
    mask_e = nc.dram_tensor("mask", [128, NS + (DIAG - 1) * 128], BF16,
                            kind="ExternalInput").ap()
    # bf16 staging for the output (host casts back to f32): halves the
    # tail out-DMA bytes; adds ~0.4% element-wise rounding, well inside
    # the error budget
    out_e = nc.dram_tensor("out", [NB * OW, D], BF16,
                           kind="ExternalOutput").ap()

    # a2a payload: per-dest block = [128 e-rows, HG*SW] from each source
    # core (source (b, r) holds heads r*HQL..r*HQL+HQL-1 of batch b).
    # Each strip's exchange is split into two head-half collectives so
    # the first can rendezvous + transfer while the second half's
    # attention passes still run.
    HG = 1                  # heads per collective
    NG = HQL // HG          # collectives per strip
    a2a_in = [[nc.dram_tensor(f"a2a_in{m}_{g}",
                              [N_CORES * 128, HG * SW], BF16)
               for g in range(NG)] for m in range(NM)]
    a2a_out = [[nc.dram_tensor(f"a2a_out{m}_{g}",
                               [N_CORES * 128, HG * SW], BF16)
                for g in range(NG)] for m in range(NM)]
    groups = [list(range(N_CORES))]

    with tile.TileContext(nc) as tc, ExitStack() as ctx:
        ep = ctx.enter_context
        const_pool = ep(tc.tile_pool(name="const", bufs=1))
        rt_pool = ep(tc.tile_pool(name="rt", bufs=HQL + 1))
        vnat_pool = ep(tc.tile_pool(name="vnat", bufs=1))
        vst_pool = ep(tc.tile_pool(name="vst", bufs=2))
        xt_pool = ep(tc.tile_pool(name="xt", bufs=2))
        wq_pool = ep(tc.tile_pool(name="wq", bufs=1))
        wkv_pool = ep(tc.tile_pool(name="wkv", bufs=1))
        wo_pool = ep(tc.tile_pool(name="wo", bufs=1))
        stg_pool = ep(tc.tile_pool(name="stg", bufs=3))
        sw_pool = ep(tc.tile_pool(name="sw", bufs=3))
        rtmp_pool = ep(tc.tile_pool(name="rtmp", bufs=4))
        pt_pool = ep(tc.tile_pool(name="pt", bufs=5))
        den_pool = ep(tc.tile_pool(name="den", bufs=2))
        rc_pool = ep(tc.tile_pool(name="rc", bufs=2))
        rbc_pool = ep(tc.tile_pool(name="rbc", bufs=2))
        attn_pool = ep(tc.tile_pool(name="attn", bufs=3))
        ao_pool = ep(tc.tile_pool(name="ao", bufs=2 * TP * 2))
        osb_pool = ep(tc.tile_pool(name="osb", bufs=3))
        # PSUM: 8 banks of [128, 512]f32. proj/outproj 2 (disjoint in
        # time) + score/transpose 2 + attention-out 2 + den-sum 2 =
        # 16KB/partition exactly.
        proj_ps = ep(tc.tile_pool(name="pops", bufs=2, space="PSUM"))
        op_ps = proj_ps
        sc_ps = ep(tc.tile_pool(name="scps", bufs=3, space="PSUM"))
        at_ps_pool = ep(tc.tile_pool(name="atps", bufs=2, space="PSUM"))
        dps_pool = ep(tc.tile_pool(name="dps", bufs=1, space="PSUM"))

        # ---- constants ----
        ident = const_pool.tile([128, 128], BF16, tag="ident")
        make_identity(nc, ident[:])
        ones = const_pool.tile([128, 2], BF16, tag="ones")
        nc.gpsimd.memset(ones[:], 1.0)
        cc = const_pool.tile([128, S], BF16, tag="cc")
        ss = const_pool.tile([128, S], BF16, tag="ss")
        msk = const_pool.tile([128, NS + (DIAG - 1) * 128], BF16, tag="msk")

        def dma_rope_consts():
            for q in range(4):
                w = S // 4
                nc.sync.dma_start(cc[:, q * w:(q + 1) * w],
                                  cc_e[:, q * w:(q + 1) * w])
                nc.sync.dma_start(ss[:, q * w:(q + 1) * w],
                                  ss_e[:, q * w:(q + 1) * w])
            nc.sync.dma_start(msk[:], mask_e[:])

        # ---- weight prefetch (sync queue, one big DMA per tensor) ----
        xts = {}  # (d, j) -> AP view

        NSPLIT = max(1, ND // 8)  # d-tiles per DMA: parallel DMA engines

        def dma_x_chunk(j):
            t = xt_pool.tile([128, ND, NS], BF16, tag="xt",
                             name=f"xc{j}")
            for d0 in range(0, ND, NSPLIT):
                d1 = d0 + NSPLIT
                nc.sync.dma_start(t[:, d0:d1, :],
                                  xT_e[:, d0:d1, j * NS:(j + 1) * NS])
            for d in range(ND):
                xts[(d, j)] = t[:, d, :]

        def dma_w(dst, src, width, nsp=4):
            step = (width + nsp - 1) // nsp
            for c0 in range(0, width, step):
                c1 = min(width, c0 + step)
                nc.sync.dma_start(dst[:, c0:c1], src[:, c0:c1])

        wk_all = wkv_pool.tile([128, ND * hd], BF16, tag="wk")
        dma_w(wk_all, wkT_e, ND * hd)
        wks = [wk_all[:, d * hd:(d + 1) * hd] for d in range(ND)]
        dma_x_chunk(0)
        dma_rope_consts()
        wv_all = wkv_pool.tile([128, ND * hd], BF16, tag="wv")
        dma_w(wv_all, wvT_e, ND * hd)
        wvs = [wv_all[:, d * hd:(d + 1) * hd] for d in range(ND)]
        wq_all = wq_pool.tile([128, ND * MQ], BF16, tag="wq")
        dma_w(wq_all, wqT_e, ND * MQ, nsp=8)
        wqs = [wq_all[:, d * MQ:(d + 1) * MQ] for d in range(ND)]
        dma_x_chunk(1)

        # persistent attention operand tiles
        rts = [rt_pool.tile([128, S], BF16, tag="rt", name=f"rtq{h}")
               for h in range(HQL)]
        krt = rt_pool.tile([128, S], BF16, tag="rt", name="rtk")
        vnat = vnat_pool.tile([128, S], BF16, tag="vnat")
        wo_all = wo_pool.tile([128, NH * D], BF16, tag="wo")
        wo_tiles = [wo_all[:, ht * D:(ht + 1) * D] for ht in range(NH)]

        # ---- emitters ----
        def rope_from_ps(ps, dst, ssl):
            """RoPE a projected [128, NS] psum tile into dst[:, ssl] (bf16).

            Rows packed per 32-quadrant: [even pairs 16b..16b+15,
            odd pairs 16b..16b+15]; swap = quadrant half-rotation.
            """
            stg = stg_pool.tile([128, NS], BF16, tag="stg")
            nc.scalar.copy(stg[:], ps[:])
            sw = sw_pool.tile([128, NS], BF16, tag="sw")
            nc.vector.stream_shuffle(sw[:], stg[:], SWAP16)
            t1 = rtmp_pool.tile([128, NS], BF16, tag="rtmp")
            nc.vector.tensor_mul(t1[:], stg[:], cc[:, ssl])
            t2 = rtmp_pool.tile([128, NS], BF16, tag="rtmp")
            nc.vector.tensor_mul(t2[:], sw[:], ss[:, ssl])
            nc.vector.tensor_add(dst[:, ssl], t1[:], t2[:])

        def proj_tile_emitters(lhs_tiles, mslice, j, kind, dst):
            """One [128, NS] projection tile split into 4-matmul quarter
            emitters + a tail (rope/copy), so filler interleaves finely
            and the Act exp stream never starves behind a PE block."""
            ssl = slice(j * NS, (j + 1) * NS)
            state = {}

            def quarter(d0):
                if 'ps' not in state:
                    state['ps'] = proj_ps.tile([128, NS], F32,
                                               tag="pops", name="psp")
                ps = state['ps']
                for d in range(d0, min(d0 + 4, ND)):
                    nc.tensor.matmul(ps[:], lhs_tiles[d][:, mslice],
                                     xts[(d, j)][:, :],
                                     start=(d == 0), stop=(d == ND - 1))

            def tail():
                # staging copies stay on Act: GPSIMD cannot access PSUM
                ps = state['ps']
                if kind == "v":
                    vst = vst_pool.tile([128, NS], BF16, tag="vst")
                    nc.scalar.copy(vst[:], ps[:])
                    for t in range(DIAG):
                        tpp = sc_ps.tile([128, 128], BF16, tag="scps",
                                         name="pst")
                        nc.tensor.transpose(
                            tpp[:], vst[:, t * 128:(t + 1) * 128],
                            ident[:])
                        nc.scalar.copy(
                            vnat[:, (j * DIAG + t) * 128:
                                 (j * DIAG + t + 1) * 128], tpp[:])
                else:
                    rope_from_ps(ps, dst, ssl)

            ems = [lambda d0=d0: quarter(d0) for d0 in range(0, ND, 4)]
            ems.append(tail)
            return ems

        def proj_chunk_emitters(j):
            ems = []
            if j >= 1 and j + 1 < NC:
                ems.append(lambda j=j: dma_x_chunk(j + 1))
            ems += proj_tile_emitters(wks, slice(0, hd), j, "k", krt)
            ems += proj_tile_emitters(wvs, slice(0, hd), j, "v", None)
            for h in range(HQL):
                ems += proj_tile_emitters(
                    wqs, slice(h * hd, (h + 1) * hd), j, "q", rts[h])
            return ems

        def wo_dma_emitters():
            def one(q):  # eighth-sized DMAs: parallel engines + pipelining
                w = NH * D // 8
                nc.sync.dma_start(wo_all[:, q * w:(q + 1) * w],
                                  woT_e[:, q * w:(q + 1) * w])
            return [lambda q=q: one(q) for q in range(8)]

        # ---- attention ----
        class Pass:
            def __init__(self, j, h):
                self.j = j
                self.h = h
                self.nsk = (j + 1) * DIAG
                self.at_ps = None
                self.den = None

        def score_stage(p, si):
            # Diagonal tiles only have valid data for q >= o; narrow all
            # work (score/exp/mask/den/AV) to the live column range.
            o = max(0, si * 128 - p.j * NS)
            w = NS - o
            sc = sc_ps.tile([128, NS], F32, tag="scps", name="psc")
            nc.tensor.matmul(
                sc[:, 0:w], krt[:, si * 128:(si + 1) * 128],
                rts[p.h][:, p.j * NS + o:(p.j + 1) * NS],
                start=True, stop=True)
            pt = pt_pool.tile([128, NS], BF16, tag="pt")
            nc.scalar.activation(
                pt[:, 0:w], sc[:, 0:w],
                mybir.ActivationFunctionType.Exp, scale=scale)
            if si * 128 - p.j * NS >= 0:  # diagonal block: causal mask
                nc.vector.tensor_mul(
                    pt[:, 0:w], pt[:, 0:w], msk[:, NS - 128:2 * NS - 128 - o])
            return pt

        def denav_stage(p, si, pt):
            o = max(0, si * 128 - p.j * NS)
            w = NS - o
            if p.at_ps is None:
                p.at_ps = at_ps_pool.tile(
                    [128, NS], F32, tag="atps",
                    name=f"atps_j{p.j}_h{p.h}")
            # denominator accumulates on the DVE in bf16 (PE stays free);
            # one ones-matmul per pass does the partition sum
            if p.den is None:
                p.den = den_pool.tile([128, NS], BF16, tag="den",
                                      name=f"den_j{p.j}_h{p.h}")
                nc.vector.tensor_copy(p.den[:], pt[:])
            else:
                nc.vector.tensor_add(p.den[:, o:NS], p.den[:, o:NS],
                                     pt[:, 0:w])
            last = si == p.nsk - 1
            nc.tensor.matmul(
                p.at_ps[:, o:NS], vnat[:, si * 128:(si + 1) * 128],
                pt[:, 0:w], start=(si == 0), stop=last)

        def finalize_pass(p):
            # runs 2 pipeline steps after the pass's last denav so the
            # PE-queue den matmul never waits on the DVE's den backlog
            dps = dps_pool.tile([1, NS], F32, tag="dps",
                                name=f"dps_j{p.j}_h{p.h}")
            nc.tensor.matmul(dps[:], ones[:, 0:1], p.den[:],
                             start=True, stop=True)
            rc = rc_pool.tile([1, NS], F32, tag="rc")
            nc.vector.reciprocal_approx_fast(out=rc[:], in_=dps[:])
            rbc = rbc_pool.tile([128, NS], F32, tag="rbc")
            nc.gpsimd.partition_broadcast(rbc[:], rc[:])
            p.rbc = rbc

        def normalize_export(p):
            asb = attn_pool.tile([128, NS], BF16, tag="attn")
            nc.vector.tensor_mul(asb[:], p.at_ps[:], p.rbc[:])
            g, hh = p.h // HG, p.h % HG
            for i in range(NS // SW):
                c = p.j * NS + i * SW
                dd = (c // SW) % N_CORES
                m = c // (N_CORES * SW)
                nc.gpsimd.dma_start(
                    a2a_in[m][g].ap()[dd * 128:(dd + 1) * 128,
                                      hh * SW:(hh + 1) * SW],
                    asb[:, i * SW:(i + 1) * SW])

        def do_a2a(m, g):
            # the CC transfer runs async on the CC cores; the Pool queue
            # keeps flowing past this instruction
            nc.gpsimd.collective_compute(
                "AllToAll", mybir.AluOpType.bypass,
                ins=[a2a_in[m][g].ap().opt()],
                outs=[a2a_out[m][g].ap().opt()],
                replica_groups=groups)

        # ---- output projection ----
        def outproj_emitters(m):
            ems = []

            def mk(m, beta):
                state = {'pso': {}}

                def do_imports():
                    # aos[(h, r)]: head-dim block of global head r*HQL+h
                    aos = {}
                    for g in range(NG):
                        for r in range(TP):
                            src = beta * TP + r
                            t = ao_pool.tile(
                                [128, HG * SW], BF16, tag="ao",
                                name=f"ao_{m}_{beta}_{r}_{g}")
                            nc.sync.dma_start(
                                t[:], a2a_out[m][g].ap()
                                [src * 128:(src + 1) * 128, :])
                            for hh in range(HG):
                                aos[(g * HG + hh, r)] = \
                                    t[:, hh * SW:(hh + 1) * SW]
                    state['aos'] = aos

                def part(n, hs):
                    # matmuls for heads hs of output group n (h-major so
                    # early heads' work overlaps later heads' collectives)
                    if n not in state['pso']:
                        state['pso'][n] = op_ps.tile(
                            [SW, NS], F32, tag="pops",
                            name=f"pso_{m}_{beta}_{n}")
                    pso = state['pso'][n]
                    for h in hs:
                        for r in range(TP):
                            ht = r * HQL + h
                            nc.tensor.matmul(
                                pso[:], state['aos'][(h, r)][:],
                                wo_tiles[ht][:, n * NS:(n + 1) * NS],
                                start=(h == 0 and r == 0),
                                stop=(h == HQL - 1 and r == TP - 1))

                def fin(n):
                    part(n, [HQL - 1])
                    pso = state['pso'].pop(n)
                    ob = osb_pool.tile([SW, NS], BF16, tag="osb")
                    # split the staging copy across Act and DVE and the
                    # write across two DMAs: halves the exposed drain of
                    # the final group
                    hw_ = NS // 2
                    nc.scalar.copy(ob[:, 0:hw_], pso[:, 0:hw_])
                    nc.vector.tensor_copy(ob[:, hw_:NS], pso[:, hw_:NS])
                    r0 = beta * OW + m * SW
                    c0 = n * NS
                    nc.sync.dma_start(
                        out_e[r0:r0 + SW, c0:c0 + hw_], ob[:, 0:hw_])
                    nc.sync.dma_start(
                        out_e[r0:r0 + SW, c0 + hw_:c0 + NS], ob[:, hw_:NS])

                head = list(range(HQL - 1))
                ems.append(do_imports)
                # rolling schedule, <=2 psum groups alive: part(0) part(1)
                # fin(0) part(2) fin(1) part(3) fin(2) fin(3)
                for n in range(min(2, NO)):
                    ems.append(lambda n=n: part(n, head))
                for n in range(2, NO):
                    ems.append(lambda n=n: fin(n - 2))
                    ems.append(lambda n=n: part(n, head))
                for n in range(max(0, NO - 2), NO):
                    ems.append(lambda n=n: fin(n))

            for beta in range(NB):
                mk(m, beta)
            return ems

        # ---- top-level emission schedule ----
        # preamble: only K, V, Q-h0 of chunk 0; Q-h1..h3 become chunk-0
        # filler so softmax engines start ~3 proj tiles earlier
        for em in (proj_tile_emitters(wks, slice(0, hd), 0, "k", krt)
                   + proj_tile_emitters(wvs, slice(0, hd), 0, "v", None)
                   + proj_tile_emitters(wqs, slice(0, hd), 0, "q",
                                        rts[0])):
            em()
        q0_filler = []     # (emitter, needed_before_pass_h)
        for h in range(1, HQL):
            for em in proj_tile_emitters(
                    wqs, slice(h * hd, (h + 1) * hd), 0, "q", rts[h]):
                q0_filler.append((em, h))

        pending = []

        def flush_one():
            item = pending.pop(0)
            if item[0] == 'fin':
                pp = item[1]
                finalize_pass(pp)
                normalize_export(pp)
                if pp.j % per == per - 1 and (pp.h + 1) % HG == 0:
                    do_a2a(pp.j // per, (pp.h + 1) // HG - 1)
                return
            _, pp, psi, ppt = item
            denav_stage(pp, psi, ppt)
            if psi == pp.nsk - 1:
                pending.append(('fin', pp))

        def flush_pending():
            while pending:
                flush_one()

        for j in range(NC):
            # filler work to interleave into this chunk's attention.
            # late_filler depends on a collective result: emit it only in
            # the second half of the chunk so the in-order PE queue never
            # reaches it before the AllToAll lands.
            filler = []
            late_filler = []
            if j == 0:
                filler += [em for em, _ in q0_filler]
            if j + 1 < NC:
                filler += proj_chunk_emitters(j + 1)
            if j == 0:
                filler += wo_dma_emitters()
            for m in range(NM):
                if j >= (m + 1) * per + 1:
                    late_filler += outproj_emitters(m)
            steps = HQL * (j + 1) * DIAG
            half = steps // 2
            fper = len(filler) / max(1, half)
            lper = len(late_filler) / max(1, steps - half)
            facc = 0.0
            fidx = 0
            lacc = 0.0
            lidx = 0
            step = 0
            per_h = (len(q0_filler) // max(1, HQL - 1)) if j == 0 else 0
            for h in range(HQL):
                p = Pass(j, h)
                if j == 0 and h > 0:
                    # pass h's scores read rts[h]: its projection filler
                    # must be emitted first (trace-time dep ordering)
                    while fidx < h * per_h:
                        filler[fidx]()
                        fidx += 1
                    facc = max(facc, float(fidx))
                for si in range(p.nsk):
                    pt = score_stage(p, si)
                    while len(pending) >= 2:
                        flush_one()
                    pending.append(('step', p, si, pt))
                    step += 1
                    if step <= half:
                        facc += fper
                        while fidx < facc and fidx < len(filler):
                            filler[fidx]()
                            fidx += 1
                    else:
                        lacc += lper
                        while lidx < lacc and lidx < len(late_filler):
                            late_filler[lidx]()
                            lidx += 1
            while fidx < len(filler):
                filler[fidx]()
                fidx += 1
            while lidx < len(late_filler):
                late_filler[lidx]()
                lidx += 1
        flush_pending()

        # tail: strips whose out-projection wasn't emitted as filler
        for m in range(NM):
            if not (NC - 1 >= (m + 1) * per + 1):
                for em in outproj_emitters(m):
                    em()

    nc.compile()
    return nc


def host_prepare(x, wq, wk, wv, wo, S, D, HQL, NS):
    """Layout-only host prep: slice/transpose/cast + rope tables + mask."""
    hd = HD
    MQ = HQL * hd
    bf = ml_dtypes.bfloat16
    DIAG = NS // 128

    # head-dim permutation: per 32-row quadrant b, rows [16 even pairs,
    # 16 odd pairs] so stream_shuffle's quadrant half-swap exchanges them
    perm = np.empty(hd, dtype=np.int64)
    for r in range(hd):
        b, w = r // 32, r % 32
        perm[r] = 2 * (16 * b + w) if w < 16 else 2 * (16 * b + (w - 16)) + 1

    def permute_heads(w):
        nh = w.shape[0] // hd
        w = w.reshape(nh, hd, -1)[:, perm, :]
        return w.reshape(nh * hd, -1)

    wq_p = permute_heads(wq)
    wk_p = permute_heads(wk)

    inv_freq = 1.0 / (ROPE_THETA ** (np.arange(0, hd, 2, dtype=np.float64)
                                     / hd))
    ang = np.arange(S, dtype=np.float64)[None, :] * inv_freq[:, None]
    cosf = np.cos(ang)  # [hd/2, S], row = pair index
    sinf = np.sin(ang)
    cci = np.empty((128, S), dtype=np.float64)
    ssi = np.empty((128, S), dtype=np.float64)
    for r in range(128):
        b, w = r // 32, r % 32
        pair = 16 * b + (w % 16)
        cci[r] = cosf[pair]
        ssi[r] = -sinf[pair] if w < 16 else sinf[pair]
    cci = cci.astype(bf)
    ssi = ssi.astype(bf)

    p = np.arange(128)[:, None]
    c = np.arange(NS + (DIAG - 1) * 128)[None, :]
    mski = (p <= c - (NS - 128)).astype(bf)

    def pack(wT):
        """[D, W] -> [128, (D//128)*W] partition-major (one-DMA layout)."""
        Dd, W = wT.shape
        return np.ascontiguousarray(
            wT.reshape(Dd // 128, 128, W).transpose(1, 0, 2)
            .reshape(128, -1)).astype(bf)

    woT = pack(wo.T)

    in_maps = []
    for core in range(N_CORES):
        b = core // TP
        r = core % TP
        qsl = slice(r * MQ, (r + 1) * MQ)
        ksl = slice(r * hd, (r + 1) * hd)
        in_maps.append({
            "xT": pack(x[b].T).reshape(128, D // 128, S),
            "wqT": pack(wq_p[qsl].T),
            "wkT": pack(wk_p[ksl].T),
            "wvT": pack(wv[ksl].T),
            "woT": woT,
            "cc": cci, "ss": ssi, "mask": mski,
        })
    return in_maps


_NC_CACHE = {}


def get_graph(S=2048, D=2048, HQL=4, NS=512):
    key = (S, D, HQL, NS)
    if key not in _NC_CACHE:
        _NC_CACHE[key] = build_graph(S, D, HQL, NS)
    return _NC_CACHE[key]


def unshard_out(results, B, S, D):
    """results[core]["out"] is [NB*OW, D] with rows (beta, strip m, SW)."""
    out = np.empty((B, S, D), dtype=np.float32)
    OW = S // N_CORES
    NM = max(1, S // (N_CORES * 128))
    SW = OW // NM
    for core in range(N_CORES):
        r = results[core]["out"].astype(np.float32)
        for beta in range(B):
            for m in range(NM):
                c0 = core * SW + m * N_CORES * SW
                out[beta, c0:c0 + SW, :] = \
                    r[beta * OW + m * SW:beta * OW + (m + 1) * SW, :]
    return out


def kernel(x, wq, wk, wv, wo, trace=False):
    B, S, D = x.shape
    HQL = (wq.shape[0] // HD) // TP
    NS = 512
    nc = get_graph(S, D, HQL, NS)
    in_maps = host_prepare(x, wq, wk, wv, wo, S, D, HQL, NS)
    res = run_bass_kernel_spmd(nc, in_maps, core_ids=list(range(N_CORES)),
                               trace=trace)
    out = unshard_out(res.results, B, S, D)
    if trace:
        kernel.last_exec_time_ns = res.exec_time_ns
        kernel.last_results = res
    return out


# revision 69
# speedup vs baseline: 1.0324x; 1.0225x over previous
"""Trainium2 Bass kernel for GQA causal attention (B=2, S=2048, D=2048,
16 q-heads / 4 kv-heads, head_dim=128, interleaved RoPE).

Sharding: DP=2 over batch x TP=4 over head groups (8 cores).
Core c: batch b=c//4, rank r=c%4 -> q-heads [4r,4r+4), kv-head r.

v2: chunk-major software pipeline. Projections (K,V,Q) for chunk j+1 and
the strip-0 output projection are emitted as PE filler inside attention
chunk j, so the PE never waits on softmax (Act/DVE) work. DVE load cut
~3x: rope via stream_shuffle (even/odd packed at 16-row granularity in
each 32-partition quadrant) + 3 bf16 ops; softmax denominator
accumulated in bf16 split across DVE (masked tiles) and Pool (unmasked);
reciprocal via reciprocal_approx_fast. Host-side work is layout only:
slicing, transposing, bf16 casting.
"""

import math
import sys

sys.path.insert(0, "/opt/trn_rl_repo")

from contextlib import ExitStack

import ml_dtypes
import numpy as np

import concourse.bass as bass
import concourse.mybir as mybir
import concourse.tile as tile
from concourse import bacc
from concourse.bass_utils import run_bass_kernel_spmd
from concourse.masks import make_identity

BF16 = mybir.dt.bfloat16
F32 = mybir.dt.float32

N_HEADS = 16
N_KV_HEADS = 4
HD = 128
ROPE_THETA = 10000.0
TP = 4
N_CORES = 8

SWAP16 = [(i + 16) % 32 for i in range(32)]  # quadrant half-swap


def build_graph(S=2048, D=2048, HQL=4, NS=512):
    """Per-core SPMD graph. HQL = local q heads; local kv heads = 1."""
    hd = HD
    ND = D // 128          # d-tiles (projection contraction tiles)
    NC = S // NS           # s-chunks
    NK = S // 128          # sk-tiles
    MQ = HQL * hd          # local q width
    DIAG = NS // 128       # sk-tiles per chunk needing a causal mask
    NB = N_CORES // TP     # batches
    OW = S // N_CORES      # out cols per core per batch
    NM = max(1, S // (N_CORES * 128))   # strips (AllToAll count)
    SW = OW // NM          # strip width (=128 at full size)
    per = NC // NM         # chunks per strip
    scale = 1.0 / math.sqrt(hd)
    NH = TP * HQL          # global head count
    NO = D // NS           # out-proj n tiles

    nc = bacc.Bacc("TRN2", target_bir_lowering=False, debug=False,
                   num_devices=N_CORES)

    # All weight/activation inputs are host-packed partition-major
    # ([128, tiles*width]) so each loads with one large-row DMA.
    xT_e = nc.dram_tensor("xT", [128, ND, S], BF16,
                          kind="ExternalInput").ap()
    wqT_e = nc.dram_tensor("wqT", [128, ND * MQ], BF16,
                           kind="ExternalInput").ap()
    wkT_e = nc.dram_tensor("wkT", [128, ND * hd], BF16,
                           kind="ExternalInput").ap()
    wvT_e = nc.dram_tensor("wvT", [128, ND * hd], BF16,
                           kind="ExternalInput").ap()
    woT_e = nc.dram_tensor("woT", [128, NH * D], BF16,
                           kind="ExternalInput").ap()
    cc_e = nc.dram_tensor("cc", [128, S], BF16, kind="ExternalInput").ap()
    ss_e = nc.dram_tensor("ss", [128, S], BF16, kind="ExternalInput").ap()
    mask_e = nc.dram_tensor("mask", [128, NS + (DIAG - 1) * 128], BF16,
                            kind="ExternalInput").ap()
    # bf16 staging for the output (host casts back to f32): halves the
    # tail out-DMA bytes; adds ~0.4% element-wise rounding, well inside
    # the error budget
    out_e = nc.dram_tensor("out", [NB * OW, D], BF16,
                           kind="ExternalOutput").ap()

    # a2a payload: per-dest block = [128 e-rows, HG*SW] from each source
    # core (source (b, r) holds heads r*HQL..r*HQL+HQL-1 of batch b).
    # Each strip's exchange is split into two head-half collectives so
    # the first can rendezvous + transfer while the second half's
    # attention passes still run.
    HG = 1                  # heads per collective
    NG = HQL // HG          # collectives per strip
    a2a_in = [[nc.dram_tensor(f"a2a_in{m}_{g}",
                              [N_CORES * 128, HG * SW], BF16)
               for g in range(NG)] for m in range(NM)]
    a2a_out = [[nc.dram_tensor(f"a2a_out{m}_{g}",
                               [N_CORES * 128, HG * SW], BF16)
                for g in range(NG)] for m in range(NM)]
    groups = [list(range(N_CORES))]

    with tile.TileContext(nc) as tc, ExitStack() as ctx:
        ep = ctx.enter_context
        const_pool = ep(tc.tile_pool(name="const", bufs=1))
        rt_pool = ep(tc.tile_pool(name="rt", bufs=HQL + 1))
        vnat_pool = ep(tc.tile_pool(name="vnat", bufs=1))
        vst_pool = ep(tc.tile_pool(name="vst", bufs=2))
        xt_pool = ep(tc.tile_pool(name="xt", bufs=2))
        wq_pool = ep(tc.tile_pool(name="wq", bufs=1))
        wkv_pool = ep(tc.tile_pool(name="wkv", bufs=1))
        wo_pool = ep(tc.tile_pool(name="wo", bufs=1))
        stg_pool = ep(tc.tile_pool(name="stg", bufs=3))
        sw_pool = ep(tc.tile_pool(name="sw", bufs=3))
        rtmp_pool = ep(tc.tile_pool(name="rtmp", bufs=4))
        pt_pool = ep(tc.tile_pool(name="pt", bufs=6))
        den_pool = ep(tc.tile_pool(name="den", bufs=2))
        rc_pool = ep(tc.tile_pool(name="rc", bufs=2))
        rbc_pool = ep(tc.tile_pool(name="rbc", bufs=2))
        attn_pool = ep(tc.tile_pool(name="attn", bufs=3))
        ao_pool = ep(tc.tile_pool(name="ao", bufs=2 * TP * 2))
        osb_pool = ep(tc.tile_pool(name="osb", bufs=3))
        # PSUM: 8 banks of [128, 512]f32. proj/outproj 2 (disjoint in
        # time) + score/transpose 2 + attention-out 2 + den-sum 2 =
        # 16KB/partition exactly.
        proj_ps = ep(tc.tile_pool(name="pops", bufs=2, space="PSUM"))
        op_ps = proj_ps
        # den-sum tiles share the score pool's rotation (they live ~1us
        # per pass), freeing a bank for a 3-deep score lookahead
        sc_ps = ep(tc.tile_pool(name="scps", bufs=4, space="PSUM"))
        at_ps_pool = ep(tc.tile_pool(name="atps", bufs=2, space="PSUM"))
        dps_pool = sc_ps

        # ---- constants ----
        ident = const_pool.tile([128, 128], BF16, tag="ident")
        make_identity(nc, ident[:])
        ones = const_pool.tile([128, 2], BF16, tag="ones")
        nc.gpsimd.memset(ones[:], 1.0)
        cc = const_pool.tile([128, S], BF16, tag="cc")
        ss = const_pool.tile([128, S], BF16, tag="ss")
        msk = const_pool.tile([128, NS + (DIAG - 1) * 128], BF16, tag="msk")

        def dma_rope_consts():
            for q in range(4):
                w = S // 4
                nc.sync.dma_start(cc[:, q * w:(q + 1) * w],
                                  cc_e[:, q * w:(q + 1) * w])
                nc.sync.dma_start(ss[:, q * w:(q + 1) * w],
                                  ss_e[:, q * w:(q + 1) * w])
            nc.sync.dma_start(msk[:], mask_e[:])

        # ---- weight prefetch (sync queue, one big DMA per tensor) ----
        xts = {}  # (d, j) -> AP view

        NSPLIT = max(1, ND // 8)  # d-tiles per DMA: parallel DMA engines

        def dma_x_chunk(j):
            t = xt_pool.tile([128, ND, NS], BF16, tag="xt",
                             name=f"xc{j}")
            for d0 in range(0, ND, NSPLIT):
                d1 = d0 + NSPLIT
                nc.sync.dma_start(t[:, d0:d1, :],
                                  xT_e[:, d0:d1, j * NS:(j + 1) * NS])
            for d in range(ND):
                xts[(d, j)] = t[:, d, :]

        def dma_w(dst, src, width, nsp=4):
            step = (width + nsp - 1) // nsp
            for c0 in range(0, width, step):
                c1 = min(width, c0 + step)
                nc.sync.dma_start(dst[:, c0:c1], src[:, c0:c1])

        wk_all = wkv_pool.tile([128, ND * hd], BF16, tag="wk")
        dma_w(wk_all, wkT_e, ND * hd)
        wks = [wk_all[:, d * hd:(d + 1) * hd] for d in range(ND)]
        dma_x_chunk(0)
        dma_rope_consts()
        wv_all = wkv_pool.tile([128, ND * hd], BF16, tag="wv")
        dma_w(wv_all, wvT_e, ND * hd)
        wvs = [wv_all[:, d * hd:(d + 1) * hd] for d in range(ND)]
        wq_all = wq_pool.tile([128, ND * MQ], BF16, tag="wq")
        dma_w(wq_all, wqT_e, ND * MQ, nsp=8)
        wqs = [wq_all[:, d * MQ:(d + 1) * MQ] for d in range(ND)]
        dma_x_chunk(1)

        # persistent attention operand tiles
        rts = [rt_pool.tile([128, S], BF16, tag="rt", name=f"rtq{h}")
               for h in range(HQL)]
        krt = rt_pool.tile([128, S], BF16, tag="rt", name="rtk")
        vnat = vnat_pool.tile([128, S], BF16, tag="vnat")
        wo_all = wo_pool.tile([128, NH * D], BF16, tag="wo")
        wo_tiles = [wo_all[:, ht * D:(ht + 1) * D] for ht in range(NH)]

        # ---- emitters ----
        def rope_from_ps(ps, dst, ssl):
            """RoPE a projected [128, NS] psum tile into dst[:, ssl] (bf16).

            Rows packed per 32-quadrant: [even pairs 16b..16b+15,
            odd pairs 16b..16b+15]; swap = quadrant half-rotation.
            """
            stg = stg_pool.tile([128, NS], BF16, tag="stg")
            nc.scalar.copy(stg[:], ps[:])
            sw = sw_pool.tile([128, NS], BF16, tag="sw")
            nc.vector.stream_shuffle(sw[:], stg[:], SWAP16)
            t1 = rtmp_pool.tile([128, NS], BF16, tag="rtmp")
            nc.vector.tensor_mul(t1[:], stg[:], cc[:, ssl])
            t2 = rtmp_pool.tile([128, NS], BF16, tag="rtmp")
            nc.vector.tensor_mul(t2[:], sw[:], ss[:, ssl])
            nc.vector.tensor_add(dst[:, ssl], t1[:], t2[:])

        def proj_tile_emitters(lhs_tiles, mslice, j, kind, dst):
            """One [128, NS] projection tile split into 4-matmul quarter
            emitters + a tail (rope/copy), so filler interleaves finely
            and the Act exp stream never starves behind a PE block."""
            ssl = slice(j * NS, (j + 1) * NS)
            state = {}

            def quarter(d0):
                if 'ps' not in state:
                    state['ps'] = proj_ps.tile([128, NS], F32,
                                               tag="pops", name="psp")
                ps = state['ps']
                for d in range(d0, min(d0 + 4, ND)):
                    nc.tensor.matmul(ps[:], lhs_tiles[d][:, mslice],
                                     xts[(d, j)][:, :],
                                     start=(d == 0), stop=(d == ND - 1))

            def tail():
                # staging copies stay on Act: GPSIMD cannot access PSUM
                ps = state['ps']
                if kind == "v":
                    vst = vst_pool.tile([128, NS], BF16, tag="vst")
                    nc.scalar.copy(vst[:], ps[:])
                    for t in range(DIAG):
                        tpp = sc_ps.tile([128, 128], BF16, tag="scps",
                                         name="pst")
                        nc.tensor.transpose(
                            tpp[:], vst[:, t * 128:(t + 1) * 128],
                            ident[:])
                        nc.scalar.copy(
                            vnat[:, (j * DIAG + t) * 128:
                                 (j * DIAG + t + 1) * 128], tpp[:])
                else:
                    rope_from_ps(ps, dst, ssl)

            ems = [lambda d0=d0: quarter(d0) for d0 in range(0, ND, 4)]
            ems.append(tail)
            return ems

        def proj_chunk_emitters(j):
            ems = []
            if j >= 1 and j + 1 < NC:
                ems.append(lambda j=j: dma_x_chunk(j + 1))
            ems += proj_tile_emitters(wks, slice(0, hd), j, "k", krt)
            ems += proj_tile_emitters(wvs, slice(0, hd), j, "v", None)
            for h in range(HQL):
                ems += proj_tile_emitters(
                    wqs, slice(h * hd, (h + 1) * hd), j, "q", rts[h])
            return ems

        def wo_dma_emitters():
            def one(q):  # eighth-sized DMAs: parallel engines + pipelining
                w = NH * D // 8
                nc.sync.dma_start(wo_all[:, q * w:(q + 1) * w],
                                  woT_e[:, q * w:(q + 1) * w])
            return [lambda q=q: one(q) for q in range(8)]

        # ---- attention ----
        class Pass:
            def __init__(self, j, h):
                self.j = j
                self.h = h
                self.nsk = (j + 1) * DIAG
                self.at_ps = None
                self.den = None

        def score_stage(p, si):
            # Diagonal tiles only have valid data for q >= o; narrow all
            # work (score/exp/mask/den/AV) to the live column range.
            o = max(0, si * 128 - p.j * NS)
            w = NS - o
            sc = sc_ps.tile([128, NS], F32, tag="scps", name="psc")
            nc.tensor.matmul(
                sc[:, 0:w], krt[:, si * 128:(si + 1) * 128],
                rts[p.h][:, p.j * NS + o:(p.j + 1) * NS],
                start=True, stop=True)
            pt = pt_pool.tile([128, NS], BF16, tag="pt")
            nc.scalar.activation(
                pt[:, 0:w], sc[:, 0:w],
                mybir.ActivationFunctionType.Exp, scale=scale)
            if si * 128 - p.j * NS >= 0:  # diagonal block: causal mask
                nc.vector.tensor_mul(
                    pt[:, 0:w], pt[:, 0:w], msk[:, NS - 128:2 * NS - 128 - o])
            return pt

        def denav_stage(p, si, pt):
            o = max(0, si * 128 - p.j * NS)
            w = NS - o
            if p.at_ps is None:
                p.at_ps = at_ps_pool.tile(
                    [128, NS], F32, tag="atps",
                    name=f"atps_j{p.j}_h{p.h}")
            # denominator accumulates on the DVE in bf16 (PE stays free);
            # one ones-matmul per pass does the partition sum
            if p.den is None:
                p.den = den_pool.tile([128, NS], BF16, tag="den",
                                      name=f"den_j{p.j}_h{p.h}")
                nc.vector.tensor_copy(p.den[:], pt[:])
            else:
                nc.vector.tensor_add(p.den[:, o:NS], p.den[:, o:NS],
                                     pt[:, 0:w])
            last = si == p.nsk - 1
            nc.tensor.matmul(
                p.at_ps[:, o:NS], vnat[:, si * 128:(si + 1) * 128],
                pt[:, 0:w], start=(si == 0), stop=last)

        def finalize_pass(p):
            # runs 2 pipeline steps after the pass's last denav so the
            # PE-queue den matmul never waits on the DVE's den backlog
            dps = dps_pool.tile([1, NS], F32, tag="scps",
                                name=f"dps_j{p.j}_h{p.h}")
            nc.tensor.matmul(dps[:], ones[:, 0:1], p.den[:],
                             start=True, stop=True)
            rc = rc_pool.tile([1, NS], F32, tag="rc")
            nc.vector.reciprocal_approx_fast(out=rc[:], in_=dps[:])
            rbc = rbc_pool.tile([128, NS], F32, tag="rbc")
            nc.gpsimd.partition_broadcast(rbc[:], rc[:])
            p.rbc = rbc

        def normalize_export(p):
            asb = attn_pool.tile([128, NS], BF16, tag="attn")
            nc.vector.tensor_mul(asb[:], p.at_ps[:], p.rbc[:])
            g, hh = p.h // HG, p.h % HG
            for i in range(NS // SW):
                c = p.j * NS + i * SW
                dd = (c // SW) % N_CORES
                m = c // (N_CORES * SW)
                nc.gpsimd.dma_start(
                    a2a_in[m][g].ap()[dd * 128:(dd + 1) * 128,
                                      hh * SW:(hh + 1) * SW],
                    asb[:, i * SW:(i + 1) * SW])

        def do_a2a(m, g):
            # the CC transfer runs async on the CC cores; the Pool queue
            # keeps flowing past this instruction
            nc.gpsimd.collective_compute(
                "AllToAll", mybir.AluOpType.bypass,
                ins=[a2a_in[m][g].ap().opt()],
                outs=[a2a_out[m][g].ap().opt()],
                replica_groups=groups)

        # ---- output projection ----
        def outproj_emitters(m):
            ems = []

            def mk(m, beta):
                state = {'pso': {}}

                def do_imports():
                    # aos[(h, r)]: head-dim block of global head r*HQL+h
                    aos = {}
                    for g in range(NG):
                        for r in range(TP):
                            src = beta * TP + r
                            t = ao_pool.tile(
                                [128, HG * SW], BF16, tag="ao",
                                name=f"ao_{m}_{beta}_{r}_{g}")
                            nc.sync.dma_start(
                                t[:], a2a_out[m][g].ap()
                                [src * 128:(src + 1) * 128, :])
                            for hh in range(HG):
                                aos[(g * HG + hh, r)] = \
                                    t[:, hh * SW:(hh + 1) * SW]
                    state['aos'] = aos

                def part(n, hs):
                    # matmuls for heads hs of output group n (h-major so
                    # early heads' work overlaps later heads' collectives)
                    if n not in state['pso']:
                        state['pso'][n] = op_ps.tile(
                            [SW, NS], F32, tag="pops",
                            name=f"pso_{m}_{beta}_{n}")
                    pso = state['pso'][n]
                    for h in hs:
                        for r in range(TP):
                            ht = r * HQL + h
                            nc.tensor.matmul(
                                pso[:], state['aos'][(h, r)][:],
                                wo_tiles[ht][:, n * NS:(n + 1) * NS],
                                start=(h == 0 and r == 0),
                                stop=(h == HQL - 1 and r == TP - 1))

                def fin(n):
                    part(n, [HQL - 1])
                    pso = state['pso'].pop(n)
                    ob = osb_pool.tile([SW, NS], BF16, tag="osb")
                    # split the staging copy across Act and DVE and the
                    # write across two DMAs: halves the exposed drain of
                    # the final group
                    hw_ = NS // 2
                    nc.scalar.copy(ob[:, 0:hw_], pso[:, 0:hw_])
                    nc.vector.tensor_copy(ob[:, hw_:NS], pso[:, hw_:NS])
                    r0 = beta * OW + m * SW
                    c0 = n * NS
                    nc.sync.dma_start(
                        out_e[r0:r0 + SW, c0:c0 + hw_], ob[:, 0:hw_])
                    nc.sync.dma_start(
                        out_e[r0:r0 + SW, c0 + hw_:c0 + NS], ob[:, hw_:NS])

                head = list(range(HQL - 1))
                ems.append(do_imports)
                # rolling schedule, <=2 psum groups alive: part(0) part(1)
                # fin(0) part(2) fin(1) part(3) fin(2) fin(3)
                for n in range(min(2, NO)):
                    ems.append(lambda n=n: part(n, head))
                for n in range(2, NO):
                    ems.append(lambda n=n: fin(n - 2))
                    ems.append(lambda n=n: part(n, head))
                for n in range(max(0, NO - 2), NO):
                    ems.append(lambda n=n: fin(n))

            for beta in range(NB):
                mk(m, beta)
            return ems

        # ---- top-level emission schedule ----
        # preamble: only K, V, Q-h0 of chunk 0; Q-h1..h3 become chunk-0
        # filler so softmax engines start ~3 proj tiles earlier
        for em in (proj_tile_emitters(wks, slice(0, hd), 0, "k", krt)
                   + proj_tile_emitters(wvs, slice(0, hd), 0, "v", None)
                   + proj_tile_emitters(wqs, slice(0, hd), 0, "q",
                                        rts[0])):
            em()
        q0_filler = []     # (emitter, needed_before_pass_h)
        for h in range(1, HQL):
            for em in proj_tile_emitters(
                    wqs, slice(h * hd, (h + 1) * hd), 0, "q", rts[h]):
                q0_filler.append((em, h))

        pending = []

        def flush_one():
            item = pending.pop(0)
            if item[0] == 'fin':
                pp = item[1]
                finalize_pass(pp)
                normalize_export(pp)
                if pp.j % per == per - 1 and (pp.h + 1) % HG == 0:
                    do_a2a(pp.j // per, (pp.h + 1) // HG - 1)
                return
            _, pp, psi, ppt = item
            denav_stage(pp, psi, ppt)
            if psi == pp.nsk - 1:
                pending.append(('fin', pp))

        def flush_pending():
            while pending:
                flush_one()

        for j in range(NC):
            # filler work to interleave into this chunk's attention.
            # late_filler depends on a collective result: emit it only in
            # the second half of the chunk so the in-order PE queue never
            # reaches it before the AllToAll lands.
            filler = []
            late_filler = []
            if j == 0:
                filler += [em for em, _ in q0_filler]
            if j + 1 < NC:
                filler += proj_chunk_emitters(j + 1)
            if j == 0:
                filler += wo_dma_emitters()
            for m in range(NM):
                if j >= (m + 1) * per + 1:
                    late_filler += outproj_emitters(m)
            steps = HQL * (j + 1) * DIAG
            half = steps // 2
            fper = len(filler) / max(1, half)
            lper = len(late_filler) / max(1, steps - half)
            facc = 0.0
            fidx = 0
            lacc = 0.0
            lidx = 0
            step = 0
            per_h = (len(q0_filler) // max(1, HQL - 1)) if j == 0 else 0
            for h in range(HQL):
                p = Pass(j, h)
                if j == 0 and h > 0:
                    # pass h's scores read rts[h]: its projection filler
                    # must be emitted first (trace-time dep ordering)
                    while fidx < h * per_h:
                        filler[fidx]()
                        fidx += 1
                    facc = max(facc, float(fidx))
                for si in range(p.nsk):
                    pt = score_stage(p, si)
                    while len(pending) >= 3:
                        flush_one()
                    pending.append(('step', p, si, pt))
                    step += 1
                    if step <= half:
                        facc += fper
                        while fidx < facc and fidx < len(filler):
                            filler[fidx]()
                            fidx += 1
                    else:
                        lacc += lper
                        while lidx < lacc and lidx < len(late_filler):
                            late_filler[lidx]()
                            lidx += 1
            while fidx < len(filler):
                filler[fidx]()
                fidx += 1
            while lidx < len(late_filler):
                late_filler[lidx]()
                lidx += 1
        flush_pending()

        # tail: strips whose out-projection wasn't emitted as filler
        for m in range(NM):
            if not (NC - 1 >= (m + 1) * per + 1):
                for em in outproj_emitters(m):
                    em()

    nc.compile()
    return nc


def host_prepare(x, wq, wk, wv, wo, S, D, HQL, NS):
    """Layout-only host prep: slice/transpose/cast + rope tables + mask."""
    hd = HD
    MQ = HQL * hd
    bf = ml_dtypes.bfloat16
    DIAG = NS // 128

    # head-dim permutation: per 32-row quadrant b, rows [16 even pairs,
    # 16 odd pairs] so stream_shuffle's quadrant half-swap exchanges them
    perm = np.empty(hd, dtype=np.int64)
    for r in range(hd):
        b, w = r // 32, r % 32
        perm[r] = 2 * (16 * b + w) if w < 16 else 2 * (16 * b + (w - 16)) + 1

    def permute_heads(w):
        nh = w.shape[0] // hd
        w = w.reshape(nh, hd, -1)[:, perm, :]
        return w.reshape(nh * hd, -1)

    wq_p = permute_heads(wq)
    wk_p = permute_heads(wk)

    inv_freq = 1.0 / (ROPE_THETA ** (np.arange(0, hd, 2, dtype=np.float64)
                                     / hd))
    ang = np.arange(S, dtype=np.float64)[None, :] * inv_freq[:, None]
    cosf = np.cos(ang)  # [hd/2, S], row = pair index
    sinf = np.sin(ang)
    cci = np.empty((128, S), dtype=np.float64)
    ssi = np.empty((128, S), dtype=np.float64)
    for r in range(128):
        b, w = r // 32, r % 32
        pair = 16 * b + (w % 16)
        cci[r] = cosf[pair]
        ssi[r] = -sinf[pair] if w < 16 else sinf[pair]
    cci = cci.astype(bf)
    ssi = ssi.astype(bf)

    p = np.arange(128)[:, None]
    c = np.arange(NS + (DIAG - 1) * 128)[None, :]
    mski = (p <= c - (NS - 128)).astype(bf)

    def pack(wT):
        """[D, W] -> [128, (D//128)*W] partition-major (one-DMA layout)."""
        Dd, W = wT.shape
        return np.ascontiguousarray(
            wT.reshape(Dd // 128, 128, W).transpose(1, 0, 2)
            .reshape(128, -1)).astype(bf)

    woT = pack(wo.T)

    in_maps = []
    for core in range(N_CORES):
        b = core // TP
        r = core % TP
        qsl = slice(r * MQ, (r + 1) * MQ)
        ksl = slice(r * hd, (r + 1) * hd)
        in_maps.append({
            "xT": pack(x[b].T).reshape(128, D // 128, S),
            "wqT": pack(wq_p[qsl].T),
            "wkT": pack(wk_p[ksl].T),
            "wvT": pack(wv[ksl].T),
            "woT": woT,
            "cc": cci, "ss": ssi, "mask": mski,
        })
    return in_maps


_NC_CACHE = {}


def get_graph(S=2048, D=2048, HQL=4, NS=512):
    key = (S, D, HQL, NS)
    if key not in _NC_CACHE:
        _NC_CACHE[key] = build_graph(S, D, HQL, NS)
    return _NC_CACHE[key]


def unshard_out(results, B, S, D):
    """results[core]["out"] is [NB*OW, D] with rows (beta, strip m, SW)."""
    out = np.empty((B, S, D), dtype=np.float32)
    OW = S // N_CORES
    NM = max(1, S // (N_CORES * 128))
    SW = OW // NM
    for core in range(N_CORES):
        r = results[core]["out"].astype(np.float32)
        for beta in range(B):
            for m in range(NM):
                c0 = core * SW + m * N_CORES * SW
                out[beta, c0:c0 + SW, :] = \
                    r[beta * OW + m * SW:beta * OW + (m + 1) * SW, :]
    return out


def kernel(x, wq, wk, wv, wo, trace=False):
    B, S, D = x.shape
    HQL = (wq.shape[0] // HD) // TP
    NS = 512
    nc = get_graph(S, D, HQL, NS)
    in_maps = host_prepare(x, wq, wk, wv, wo, S, D, HQL, NS)
    res = run_bass_kernel_spmd(nc, in_maps, core_ids=list(range(N_CORES)),
                               trace=trace)
    out = unshard_out(res.results, B, S, D)
    if trace:
        kernel.last_exec_time_ns = res.exec_time_ns
        kernel.last_results = res
    return out
